# revision 17
# baseline (speedup 1.0000x reference)
"""ClassAttention kernel for 8x TRN2 NeuronCores (Bass/Tile).

Problem (hardcoded): x[16, 2049, 1024], w_qkv[3072, 1024], w_proj[1024, 1024],
b_proj[1024].  Reference computes qkv projection, class-token attention
(only query position 0 attends), projection of the class token, and returns
concat([cls_tok, x[:, 1:]], axis=1).

Only output row 0 is computed; rows 1.. are x passthrough (host, mirroring the
reference's concatenate).

Algebraic restructure (same math, far fewer FLOPs):
    q0[b]        = x[b,0] @ Wq^T                       (host, tiny)
    wfold[b,h,:] = SCALE * q0[b,h,:] @ Wk_h            (host: fold q0 into Wk)
    logits[b,h,s]= sum_d x[b,s,d] * wfold[b,h,d]       (device matmul over d)
    ex           = exp(logits)                          (device, no normalize)
    xa[b,h,d]    = sum_s ex[b,h,s] * x[b,s,d]          (device matmul over s)
    sums[b,h]    = sum_s ex[b,h,s]                     (device, f32 accum)
    -- host epilogue (q0-fold-sized, O(B*D^2)): --
    attn_x       = (xa + ex_2048 * x[:,2048]) / (sums + ex_2048)
    cls[b,he]    = attn_x[b,h,:] @ Wv_h^T    (diagonal head blocks)
    out0         = cls @ Wp^T + bias

Device handles exactly s in [0, 2048) = 16 s-tiles of 128; the s=2048
remainder row is folded in on the host (it has x and wfold).

All four matmul operands (x both layouts, wfold, exp weights) are fp8e4
(e4m3) so every matmul runs in MatmulPerfMode.DoubleRow: two 128-deep
k-tiles per instruction at 2 fp8/cycle/lane - 2x PE throughput.
exp() never overflows (logits ~ N(0,1)) so no max-shift is needed; the
softmax denominator is divided out on the host, which also absorbs the
fp8-range scaling alpha folded into wfold (undone by exp's scale arg).

Sharding: data-parallel over batch, 2 batch elements per core (8 cores).
x is shipped in natural [s,d] and transposed [d,s] layouts, each
pre-permuted on the host into the exact SBUF tile layout so every DMA is
a plain linear copy with 4KB contiguous lines.

Orchestration: DMA stream order xt(b0), xn(b0), xt(b1), xn(b1) in
512KB chunks, balancing PE load (logits in each xt phase; transposes+xa
in each xn phase) against the stream throughout. Engine programs are
emitted in data-arrival order so every stage chases its just-arrived
chunk; keep-warm matmuls fill the early DMA-chase gaps so the PE clock
ramps before the real work. Outputs ship per batch from the scalar
engine's own DMA queue (all loads strictly precede outs so completion-
semaphore recycling never makes a load wait on an output flush).
"""

import numpy as np
import ml_dtypes

BF16 = ml_dtypes.bfloat16
FP8E4 = ml_dtypes.float8_e4m3

B, S, D, H, E = 16, 2049, 1024, 16, 64
SCALE = E ** -0.5
NCORES = 8
BL = B // NCORES          # batches per core = 2
SDEV = 2048               # s rows handled on device
ST = 16                   # s-tiles of 128
DT8 = 8                   # d-tiles of 128

_cached = {}


def _kernel_body(ctx, tc):
    import concourse.bass as bass
    from concourse import mybir

    nc = tc.nc
    dt = mybir.dt
    AF = mybir.ActivationFunctionType
    DR = mybir.MatmulPerfMode.DoubleRow

    # HBM layouts (pre-permuted on host so DMAs are linear 4KB lines):
    #   xt row = (b*4 + kp)*128 + p, col = k2*2048 + s  (kp = d8 pair)
    #   xn row = b*128 + p, col = st*1024 + d
    xt_d = nc.dram_tensor("xt", (BL * 4 * 128, 2 * SDEV), dt.float8e4,
                          kind="ExternalInput").ap()
    xn_d = nc.dram_tensor("xn", (BL * 128, ST * 1024), dt.float8e4,
                          kind="ExternalInput").ap()
    wf_d = nc.dram_tensor("wf", (128, BL * 128), dt.float8e4,
                          kind="ExternalInput").ap()
    id_d = nc.dram_tensor("ident", (16, 16), dt.bfloat16,
                          kind="ExternalInput").ap()
    xa_d = nc.dram_tensor("xa", (16, BL * D), dt.bfloat16,
                          kind="ExternalOutput").ap()
    se_d = nc.dram_tensor("se", (16, BL), dt.float32,
                          kind="ExternalOutput").ap()

    cpool = ctx.enter_context(tc.tile_pool(name="const", bufs=1))
    xt_pool = ctx.enter_context(tc.tile_pool(name="xt", bufs=1))
    xn_pool = ctx.enter_context(tc.tile_pool(name="xn", bufs=1))
    sm_pool = ctx.enter_context(tc.tile_pool(name="sm", bufs=1))
    st_pool = ctx.enter_context(tc.tile_pool(name="stats", bufs=2))
    at_pool = ctx.enter_context(tc.tile_pool(name="attnT", bufs=1))
    acc_pool = ctx.enter_context(tc.tile_pool(name="acc", bufs=1))

    # PSUM: logits c0..c3 (4 banks), xa (2 banks), transposes (2 banks)
    ps_log = ctx.enter_context(tc.tile_pool(name="pslog", bufs=1, space="PSUM"))
    ps_xa = ctx.enter_context(tc.tile_pool(name="psxa", bufs=1, space="PSUM"))
    ps_tr = ctx.enter_context(tc.tile_pool(name="pstr", bufs=2, space="PSUM"))

    # --- constants ---
    wf_sb = cpool.tile([128, BL * 128], dt.float8e4, tag="wf")
    nc.sync.dma_start(wf_sb[:], wf_d)
    id_sb = cpool.tile([16, 16], dt.bfloat16, tag="ident")
    nc.sync.dma_start(id_sb[:], id_d)

    # x tiles: xt[b] = [p, k8, s], xn[b] = [p, st, d]
    xt_sb = [xt_pool.tile([128, DT8, SDEV], dt.float8e4, tag=f"xt{b}",
                          name=f"xt{b}") for b in range(BL)]
    xn_sb = [xn_pool.tile([128, ST, 1024], dt.float8e4, tag=f"xn{b}",
                          name=f"xn{b}") for b in range(BL)]

    def load_xt(b, kp):
        # one d8-pair (256 d rows) for all s: 512KB, 4KB contiguous lines
        # on BOTH sides (strided dst segments would split DMA packets and
        # cost ~25% bandwidth).
        nc.sync.dma_start(
            xt_sb[b][:, kp * 2:(kp + 1) * 2, :],
            xt_d[(b * 4 + kp) * 128:(b * 4 + kp + 1) * 128, :]
            .rearrange("p (k s) -> p k s", k=2),
        )

    def load_xn(b, st0, st1):
        nc.sync.dma_start(
            xn_sb[b][:, st0:st1, :],
            xn_d[b * 128:(b + 1) * 128, st0 * 1024:st1 * 1024]
            .rearrange("p (st d) -> p st d", st=st1 - st0),
        )

    # persistent SBUF state
    exp_sb = [sm_pool.tile([16, SDEV], dt.bfloat16, tag=f"exp{b}",
                           name=f"exp{b}") for b in range(BL)]
    atT_sb = [at_pool.tile([128, ST, 16], dt.float8e4, tag=f"atT{b}",
                           name=f"atT{b}") for b in range(BL)]
    xa_sb = acc_pool.tile([16, BL * D], dt.bfloat16, tag="xa")
    se_sb = acc_pool.tile([16, BL], dt.float32, tag="se")

    # --- PE warm-up: dense matmuls so the first logits run at full clock.
    # The clock ramp needs several microseconds of sustained PE activity;
    # extra keep-warm matmuls are also interleaved into the logits DMA-chase
    # gaps (they have no data deps, so they fill what would be idle time).
    warm_sb = cpool.tile([128, 512], dt.bfloat16, tag="warm")
    nc.vector.memset(warm_sb[:], 0.0)
    _warm_n = [0]

    def emit_warm(n):
        for _ in range(n):
            w = _warm_n[0]
            _warm_n[0] += 1
            ps = ps_tr.tile([128, 512], dt.float32, tag="tr", name=f"warm{w}")
            nc.tensor.matmul(ps[:], warm_sb[:, :128], warm_sb[:], start=True,
                             stop=True)

    emit_warm(4)

    # --- DMA program order (= sync-queue FIFO order) ---
    # xn(b0) is split around xt(b1): b1's logits/exp/transpose chain then
    # starts ~2.6us earlier and completes under the xn phases, while
    # xa(b0)'s first half still gets early data. xa(b1) rides the tail.
    for kp in range(4):
        load_xt(0, kp)
    load_xn(0, 0, 4)
    load_xn(0, 4, 8)
    for kp in range(4):
        load_xt(1, kp)
    load_xn(0, 8, 12)
    load_xn(0, 12, 16)
    for st0 in range(0, 12, 4):
        load_xn(1, st0, st0 + 4)
    load_xn(1, 12, 14)
    load_xn(1, 14, 16)

    log_ch = {}

    def make_chunks(b):
        log_ch[b] = [ps_log.tile([16, 512], dt.float32, tag=f"c{c}",
                                 name=f"c{c}_{b}") for c in range(4)]

    def emit_logits_dd(b, dd):
        # logits[h, s] = sum_d wf[d, h] x^T[d, s]; DoubleRow over d8 pairs,
        # dd-outer so each group chases its just-arrived xt chunk.
        lhs = (wf_sb[:, b * 128 + dd * 32: b * 128 + (dd + 1) * 32]
               .rearrange("p (two h) -> p two h", two=2))
        for c in range(4):
            nc.tensor.matmul(
                log_ch[b][c][:],
                lhs,
                xt_sb[b][:, dd * 2:(dd + 1) * 2, c * 512:(c + 1) * 512],
                start=(dd == 0), stop=(dd == 3), perf_mode=DR,
            )

    def emit_exp(b):
        # logits ~ N(0,1): exp cannot overflow fp32; normalization happens on
        # the host, so emit raw exp with f32 row-sums. scale undoes ALPHA.
        sums = st_pool.tile([16, 4], dt.float32, tag="sums", name=f"sums{b}")
        for c in range(4):
            nc.scalar.activation(exp_sb[b][:, c * 512:(c + 1) * 512],
                                 log_ch[b][c][:], AF.Exp,
                                 bias=0.0, scale=1.0 / ALPHA,
                                 accum_out=sums[:, c: c + 1])
        nc.vector.tensor_reduce(se_sb[:, b: b + 1], sums[:],
                                axis=mybir.AxisListType.X,
                                op=mybir.AluOpType.add)

    def emit_tr_group(b, g):
        # [16,128] slices of exp -> [128,16] fp8 columns of atT, 4 per bank
        g0 = g * 4
        ps = ps_tr.tile([128, 64], dt.bfloat16, tag="tr", name=f"tr{b}_{g0}")
        for k in range(4):
            st = g0 + k
            nc.tensor.transpose(ps[:, k * 16:(k + 1) * 16],
                                exp_sb[b][:, st * 128:(st + 1) * 128],
                                id_sb[:])
        nc.vector.tensor_copy(atT_sb[b][:, g0:g0 + 4, :],
                              ps[:].rearrange("p (st h) -> p st h", st=4))

    xa_ps = {}

    def make_xa(b):
        xa_ps[b] = [ps_xa.tile([16, 512], dt.float32, tag=f"xa{c}",
                               name=f"xa{c}_{b}") for c in range(2)]

    def emit_xa_stp(b, stp):
        # xa[h, d] = sum_s ex[h,s] x[s,d]; DoubleRow over st pairs
        for c in range(2):
            nc.tensor.matmul(
                xa_ps[b][c][:],
                atT_sb[b][:, stp * 2:(stp + 1) * 2, :],
                xn_sb[b][:, stp * 2:(stp + 1) * 2, c * 512:(c + 1) * 512],
                start=(stp == 0), stop=(stp == 7), perf_mode=DR,
            )

    def emit_xa_out(b):
        # split the PSUM->SBUF evacuation across scalar+vector, then ship
        # the two 16KB halves concurrently from the scalar and sync DMA
        # queues (all loads were emitted first, so semaphore recycling
        # cannot make a load wait on these).
        nc.scalar.copy(xa_sb[:, b * D: b * D + 512], xa_ps[b][0][:])
        nc.scalar.dma_start(xa_d[:, b * D: b * D + 512],
                            xa_sb[:, b * D: b * D + 512])
        nc.vector.tensor_copy(xa_sb[:, b * D + 512:(b + 1) * D], xa_ps[b][1][:])
        nc.sync.dma_start(xa_d[:, b * D + 512:(b + 1) * D],
                          xa_sb[:, b * D + 512:(b + 1) * D])

    # --- emission in data-arrival order ---
    make_chunks(0)
    for dd in range(4):
        emit_logits_dd(0, dd)          # chases xt(b0)
        if dd < 3:
            emit_warm(2)
    emit_exp(0)
    make_xa(0)
    for g in range(2):                 # xa(b0) first half on early xn(b0)
        emit_tr_group(0, g)
        emit_xa_stp(0, 2 * g)
        emit_xa_stp(0, 2 * g + 1)
    emit_tr_group(0, 2)
    emit_tr_group(0, 3)
    make_chunks(1)
    for dd in range(4):
        emit_logits_dd(1, dd)          # chases xt(b1), now 2.6us earlier
    emit_exp(1)                        # b1 chain under the xn phases
    for stp in range(4, 8):            # xa(b0) second half
        emit_xa_stp(0, stp)
    emit_xa_out(0)
    make_xa(1)
    for g in range(4):                 # xa(b1) rides the xn(b1) tail
        emit_tr_group(1, g)
        emit_xa_stp(1, 2 * g)
        emit_xa_stp(1, 2 * g + 1)
    emit_xa_out(1)
    nc.scalar.dma_start(se_d, se_sb[:])


ALPHA = None  # set by _host_prep before _build


def _build():
    if "nc" in _cached:
        return _cached["nc"]
    from contextlib import ExitStack
    import concourse.tile as tile
    from concourse import bacc

    nc = bacc.Bacc("TRN2", target_bir_lowering=False, debug=False,
                   num_devices=NCORES)
    with tile.TileContext(nc) as tc:
        with ExitStack() as ctx:
            _kernel_body(ctx, tc)
    nc.compile()
    _cached["nc"] = nc
    return nc


def _host_prep(x, w_qkv, w_proj, b_proj):
    global ALPHA
    x = np.asarray(x, dtype=np.float32)
    w_qkv = np.asarray(w_qkv, dtype=np.float32)

    w_q, w_k = w_qkv[:D], w_qkv[D:2 * D]
    q0 = x[:, 0, :] @ w_q.T                                   # [B, D]
    wfold = np.einsum("bhe,hed->bhd", q0.reshape(B, H, E),
                      w_k.reshape(H, E, D)) * SCALE           # [B, H, D]
    # fp8e4 range scaling, undone by exp's scale argument on device
    ALPHA = float(2.0 ** np.floor(np.log2(64.0 / np.abs(wfold).max())))

    # wf core layout: [p, b*128 + d8*16 + h]
    wfT = np.ascontiguousarray(wfold.transpose(0, 2, 1))      # [B, D, H]
    id_dev = np.eye(16, dtype=BF16)

    in_maps = []
    for c in range(NCORES):
        b0 = c * BL
        xb = x[b0:b0 + BL, :SDEV]                             # [BL, 2048, 1024]
        # xn: [b, p, st, d]
        xn = np.ascontiguousarray(
            xb.reshape(BL, ST, 128, 1024).transpose(0, 2, 1, 3)
        ).astype(FP8E4)
        # xt: [b, kp, p, k2, s] -> rows (b, kp, p), cols (k2, s): 4KB lines
        xt = np.ascontiguousarray(
            xb.transpose(0, 2, 1).reshape(BL, 4, 2, 128, SDEV)
            .transpose(0, 1, 3, 2, 4)
        ).astype(FP8E4)
        wf_core = (wfT[b0:b0 + BL].reshape(BL, DT8, 128, H)
                   .transpose(2, 0, 1, 3).reshape(128, BL * 128)
                   * ALPHA).astype(FP8E4)
        in_maps.append({
            "xt": xt.reshape(BL * 4 * 128, 2 * SDEV),
            "xn": xn.reshape(BL * 128, ST * 1024),
            "wf": np.ascontiguousarray(wf_core),
            "ident": id_dev,
        })
    return x, wfold, in_maps


def _epilogue(x, wfold, w_qkv, w_proj, b_proj, xa_all, se_all):
    """Host tail: fold s=2048, normalize, project. O(B*D^2), like the q0 fold."""
    w_v = w_qkv[2 * D:].reshape(H, E, D)
    x_last = x[:, SDEV, :]                                    # [B, D]
    l_last = np.einsum("bhd,bd->bh", wfold, x_last)           # exact f32
    e_last = np.exp(l_last)                                   # [B, H]
    xa = xa_all + e_last[:, :, None] * x_last[:, None, :]     # [B, H, D]
    sums = se_all + e_last
    attn_x = xa / sums[:, :, None]
    cls = np.einsum("bhd,hed->bhe", attn_x, w_v).reshape(B, D)
    return cls @ w_proj.T + b_proj                            # [B, D]


def _run(x, w_qkv, w_proj, b_proj, trace=False):
    from concourse import bass_utils
    try:
        import jax
        jax.config.update("jax_compilation_cache_dir", "/tmp/jax_pjrt_cache")
        jax.config.update("jax_persistent_cache_min_compile_time_secs", 2.0)
    except Exception:
        pass

    x, wfold, in_maps = _host_prep(x, w_qkv, w_proj, b_proj)
    nc = _build()
    res = bass_utils.run_bass_kernel_spmd(
        nc, in_maps, core_ids=list(range(NCORES)), trace=trace)

    xa_all = np.empty((B, H, D), np.float32)
    se_all = np.empty((B, H), np.float32)
    for c in range(NCORES):
        xa_all[c * BL:(c + 1) * BL] = np.asarray(
            res.results[c]["xa"]).astype(np.float32).reshape(
                H, BL, D).transpose(1, 0, 2)
        se_all[c * BL:(c + 1) * BL] = np.asarray(
            res.results[c]["se"], dtype=np.float32).T

    w_qkv = np.asarray(w_qkv, dtype=np.float32)
    w_proj = np.asarray(w_proj, dtype=np.float32)
    b_proj = np.asarray(b_proj, dtype=np.float32)
    out0 = _epilogue(x, wfold, w_qkv, w_proj, b_proj, xa_all, se_all)

    out = x.copy()
    out[:, 0, :] = out0
    return out, res


def kernel(x, w_qkv, w_proj, b_proj):
    out, _ = _run(x, w_qkv, w_proj, b_proj, trace=False)
    return out
